# revision 7
# baseline (speedup 1.0000x reference)
"""Trainium2 Bass kernel for nn_Net_76270029242478 (gnn_message_passing).

Math (B=32, N=100, E=256, H=1024, MID=256):
  t        = einsum('bije,em->bijm', trans_mat, W_r) + b_r
  qp       = q @ W_q + b_q
  relation = einsum('bijm,m->bij', t * qp[:,None,None,:], W_out[:,0]) + b_out
  relation = where(r_mask==0, -inf, relation); softmax over i (axis=1)
  out      = einsum('bij,bj->bi', softmax, z_logits)

Algebraic fold used here (exact):
  relation[b,i,j] = trans_mat[b,i,j,:] . u[b,:] + c[b]
    u[b,e] = sum_m W_r[e,m] * qp[b,m] * W_out[m,0]
    c[b]   = sum_m b_r[m] * qp[b,m]*W_out[m,0] + b_out[0]
  c[b] is constant over (i,j) so it cancels in the softmax over i ->
  skip c / b_r / b_out entirely. This turns the 42-GFLOP einsum into a
  memory-bound streaming dot product over trans_mat.

Sharding: data-parallel over batch, 4 samples per core x 8 cores.
"""

import numpy as np

import concourse.bass as bass
import concourse.tile as tile
from concourse import bacc, mybir
from concourse.bass_utils import run_bass_kernel_spmd

F32 = mybir.dt.float32
I32 = mybir.dt.int32
Alu = mybir.AluOpType
ActF = mybir.ActivationFunctionType

B, N, E, H, MID = 32, 100, 256, 1024, 256
NCORES = 8
BPC = B // NCORES       # samples per core = 4
IBLK = 10               # i-rows per streamed trans tile
NBLK = N // IBLK        # 10
HK = H // 128           # 8 contraction chunks for q @ W_q
MK = MID // 128         # 2 contraction chunks for v @ W_r^T


def _build():
    nc = bacc.Bacc("TRN2", target_bir_lowering=False, debug=False,
                   num_devices=NCORES)

    trans_d = nc.declare_dram_parameter("trans", [BPC, N, N, E], F32, isOutput=False)
    qT_d = nc.declare_dram_parameter("qT", [H, BPC], F32, isOutput=False)
    Wq_d = nc.declare_dram_parameter("W_q", [H, MID], F32, isOutput=False)
    bq_d = nc.declare_dram_parameter("b_q", [MID, 1], F32, isOutput=False)
    Wout_d = nc.declare_dram_parameter("W_out", [MID, 1], F32, isOutput=False)
    WrT_d = nc.declare_dram_parameter("W_rT", [MID, E], F32, isOutput=False)
    maskT_d = nc.declare_dram_parameter("r_maskT", [BPC, N, N], I32, isOutput=False)
    zT_d = nc.declare_dram_parameter("zT", [N, BPC], F32, isOutput=False)
    outT_d = nc.declare_dram_parameter("outT", [N, BPC], F32, isOutput=True)

    with tile.TileContext(nc) as tc, \
         tc.tile_pool(name="const", bufs=1) as const_pool, \
         tc.tile_pool(name="stream", bufs=4) as stream_pool, \
         tc.tile_pool(name="epi", bufs=2) as epi_pool, \
         tc.tile_pool(name="psum", bufs=2, space="PSUM") as psum_pool, \
         tc.tile_pool(name="psum_big", bufs=2, space="PSUM") as psum_big:

        # ---------- weights / small inputs to SBUF ----------
        qT_sb = const_pool.tile([128, HK, BPC], F32)
        nc.sync.dma_start(qT_sb[:], qT_d[:].rearrange("(k p) b -> p k b", p=128))
        Wq_sb = const_pool.tile([128, HK, MID], F32)
        nc.sync.dma_start(Wq_sb[:], Wq_d[:].rearrange("(k p) m -> p k m", p=128))
        bq_sb = const_pool.tile([128, MK], F32)
        nc.sync.dma_start(bq_sb[:], bq_d[:].rearrange("(k p) one -> p (k one)", p=128))
        Wout_sb = const_pool.tile([128, MK], F32)
        nc.sync.dma_start(Wout_sb[:], Wout_d[:].rearrange("(k p) one -> p (k one)", p=128))
        WrT_sb = const_pool.tile([128, MK, E], F32)
        nc.sync.dma_start(WrT_sb[:], WrT_d[:].rearrange("(k p) e -> p k e", p=128))
        zT_sb = const_pool.tile([N, BPC], F32)
        nc.sync.dma_start(zT_sb[:], zT_d[:])

        ones_sb = const_pool.tile([1, N], F32)
        nc.gpsimd.memset(ones_sb[:], 1.0)

        # ---------- precompute u[b,:] and its 100-partition broadcast ----------
        # qpT[m,b] = sum_h W_q[h,m] * q[b,h]
        vT_sb = const_pool.tile([128, MK, BPC], F32)
        for mk in range(MK):
            qpT_ps = psum_pool.tile([128, BPC], F32)
            for hk in range(HK):
                nc.tensor.matmul(
                    qpT_ps[:],
                    Wq_sb[:, hk, mk * 128:(mk + 1) * 128],
                    qT_sb[:, hk, :],
                    start=(hk == 0), stop=(hk == HK - 1),
                )
            # vT[m,b] = (qpT[m,b] + b_q[m]) * W_out[m]
            nc.vector.tensor_scalar(
                out=vT_sb[:, mk, :], in0=qpT_ps[:],
                scalar1=bq_sb[:, mk:mk + 1], scalar2=Wout_sb[:, mk:mk + 1],
                op0=Alu.add, op1=Alu.mult,
            )

        # u[b,e] = sum_m vT[m,b] * W_rT[m,e], stored as one row (1, BPC*E)
        # at partition 0 so slices are valid matmul operands.
        u_flat = const_pool.tile([1, BPC * E], F32)
        for b in range(BPC):
            u_ps = psum_pool.tile([1, E], F32)
            for mk in range(MK):
                nc.tensor.matmul(
                    u_ps[:], vT_sb[:, mk, b:b + 1], WrT_sb[:, mk, :],
                    start=(mk == 0), stop=(mk == MK - 1),
                )
            nc.scalar.copy(u_flat[:, b * E:(b + 1) * E], u_ps[:])

        # uRep[b] = broadcast of u[b,:] over 100 partitions
        uRep_sb = []
        for b in range(BPC):
            uRep_ps = psum_big.tile([N, E], F32)
            nc.tensor.matmul(uRep_ps[:], ones_sb[:], u_flat[:, b * E:(b + 1) * E],
                             start=True, stop=True)
            uRep = const_pool.tile([N, E], F32, name=f"uRep{b}")
            nc.scalar.copy(uRep[:], uRep_ps[:])
            uRep_sb.append(uRep)

        # ---------- prefetch masks and build maskadd for all b upfront ----------
        maskadd_sb = []
        for b in range(BPC):
            mask_i = epi_pool.tile([N, N], I32)
            nc.scalar.dma_start(mask_i[:], maskT_d[b])
            mask_f = epi_pool.tile([N, N], F32)
            nc.gpsimd.tensor_copy(mask_f[:], mask_i[:])
            # maskadd = mask * 1e30 - 1e30  in {0, -1e30}
            maskadd = const_pool.tile([N, N], F32, name=f"maskadd{b}")
            nc.gpsimd.tensor_scalar(
                out=maskadd[:], in0=mask_f[:],
                scalar1=1.0e30, scalar2=-1.0e30,
                op0=Alu.mult, op1=Alu.add,
            )
            maskadd_sb.append(maskadd)

        # ---------- main stream: rel[b][j,i] = trans[b,i,j,:] . u[b,:] ----------
        rel_sb = const_pool.tile([N, BPC, N], F32)
        ttr_scratch = const_pool.tile([N, E], F32)
        outT_sb = const_pool.tile([N, BPC], F32)

        def stream(b):
            for blk in range(NBLK):
                tt = stream_pool.tile([N, IBLK, E], F32)
                dma_eng = nc.sync if (b * NBLK + blk) % 2 == 0 else nc.scalar
                dma_eng.dma_start(
                    tt[:],
                    trans_d[b, blk * IBLK:(blk + 1) * IBLK, :, :]
                    .rearrange("i j e -> j i e"),
                )
                for il in range(IBLK):
                    i = blk * IBLK + il
                    nc.vector.scalar_tensor_tensor(
                        out=ttr_scratch[:],
                        in0=tt[:, il, :], scalar=1.0, in1=uRep_sb[b][:],
                        op0=Alu.mult, op1=Alu.mult,
                        accum_out=rel_sb[:, b, i:i + 1],
                    )

        def epilogue(b):
            # masked softmax over i (free dim) + aggregation
            relm = epi_pool.tile([N, N], F32)
            nc.vector.tensor_add(relm[:], rel_sb[:, b, :], maskadd_sb[b][:])
            negM = epi_pool.tile([N, 1], F32)
            nc.vector.reduce_max(negM[:], relm[:], axis=mybir.AxisListType.X,
                                 negate=True)

            P_sb = epi_pool.tile([N, N], F32)
            S_sb = epi_pool.tile([N, 1], F32)
            nc.scalar.activation(P_sb[:], relm[:], ActF.Exp,
                                 bias=negM[:], scale=1.0, accum_out=S_sb[:])

            Sinv = epi_pool.tile([N, 1], F32)
            nc.vector.reciprocal(Sinv[:], S_sb[:])
            w_sb = epi_pool.tile([N, 1], F32)
            nc.vector.tensor_mul(w_sb[:], zT_sb[:, b:b + 1], Sinv[:])

            # out[i] = sum_j P[j,i] * w[j]
            o_ps = psum_pool.tile([N, 1], F32)
            nc.tensor.matmul(o_ps[:], P_sb[:], w_sb[:], start=True, stop=True)
            nc.scalar.copy(outT_sb[:, b:b + 1], o_ps[:])

        # delay each epilogue by one sample so ACT-ring stalls never gate
        # the next sample's trans DMA issues
        stream(0)
        for b in range(1, BPC):
            stream(b)
            epilogue(b - 1)
        epilogue(BPC - 1)

        nc.sync.dma_start(outT_d[:], outT_sb[:])

    nc.compile()
    return nc


_nc_cache = None


def _get_nc():
    global _nc_cache
    if _nc_cache is None:
        _nc_cache = _build()
    return _nc_cache


def _make_in_maps(q, trans_mat, r_mask, z_logits, W_r, b_r, W_q, b_q, W_out, b_out):
    in_maps = []
    W_rT = np.ascontiguousarray(W_r.T)
    b_q2 = np.ascontiguousarray(b_q.reshape(MID, 1))
    W_out2 = np.ascontiguousarray(W_out.reshape(MID, 1))
    for c in range(NCORES):
        b0 = c * BPC
        in_maps.append({
            "trans": np.ascontiguousarray(trans_mat[b0:b0 + BPC]),
            "qT": np.ascontiguousarray(q[b0:b0 + BPC].T),
            "W_q": W_q,
            "b_q": b_q2,
            "W_out": W_out2,
            "W_rT": W_rT,
            "r_maskT": np.ascontiguousarray(r_mask[b0:b0 + BPC].transpose(0, 2, 1)),
            "zT": np.ascontiguousarray(z_logits[b0:b0 + BPC].T),
        })
    return in_maps


def _run(inputs, trace=False, **kwargs):
    nc = _get_nc()
    in_maps = _make_in_maps(**inputs)
    res = run_bass_kernel_spmd(nc, in_maps, list(range(NCORES)),
                               trace=trace, **kwargs)
    out = np.empty((B, N), dtype=np.float32)
    for c in range(NCORES):
        out[c * BPC:(c + 1) * BPC, :] = np.asarray(res.results[c]["outT"]).T
    return out, res


def kernel(**inputs):
    out, _ = _run(inputs)
    return out
